# revision 8
# baseline (speedup 1.0000x reference)
"""Multi-head graph attention layer (GAT, no softmax) on 8 Trainium2 NeuronCores.

Strategy: row-shard the N=4096 nodes across the 8 cores (512 rows each).
Host precomputes Wh = h@W (bf16, [m, (h,o)] layout), the attention
projections s_h[n] = Wh.a1, t_h[m] = Wh.a2, and the transposed additive
mask bigatT[m, n] = BIG*(adj[n, m]-1) (masked entries prelu to -0.2*BIG
~= -9.007e15, matching the reference's -9e15 to ~8e-4).

Device per core, per 128-row m-block:
    P^T[m, n] = prelu_0.2(bigatT[m, n] + s[n] + t[m])   per head
    h'^T[(h,o), n] += Wh[m, (h,o)]^T @ P^T[m, n]        (PSUM accumulate)
    out = elu(h'^T)

The N^2*H elementwise work is split across three engines:
  - heads ACT_H: DVE/Pool computes xc = bigatT + s (broadcast add), ACT
    applies Prelu with per-partition bias t.
  - heads FUSED_H: one custom fused DVE op computes the whole chain
    prelu(bigatT + s + t) in a single 1-elem/cycle instruction.
"""

import numpy as np
import ml_dtypes

N = 4096
IN_F = 512
OUT_F = 64
HEADS = 8
NCORES = 8
NS = N // NCORES          # 512 rows per core
MB = N // 128             # 32 m-blocks
HO = HEADS * OUT_F        # 512
BIG = float(np.float32(1.25 * 2.0**55))   # 0.2*BIG = 2^53 ~= 9.007e15
ALPHA = 0.2

# head-assignment knobs.  Head h on m-block mb takes one of three paths:
#   "F"  — fused custom DVE op does bigat+s+t+prelu in one instruction
#   "A"  — DVE batched tensor_tensor s-add, then ACT prelu with bias=t
#   "D"  — DMA writes s-broadcast + accumulates bigat (software DGE),
#          then ACT prelu with bias=t (zero DVE cost)
import os
_NDMAS = int(os.environ.get("GAT_NDMAS", "2"))    # heads with s-add via DMA
_NFUSED = int(os.environ.get("GAT_NFUSED", "4"))  # heads on fused DVE op
_FRAC = int(os.environ.get("GAT_FRAC", "4"))      # last fused head does "A"
#                                                   every 1-in-_FRAC mb (0=off)
DMA_H = list(range(_NDMAS))                       # D-path heads
FUSED_H = list(range(HEADS - _NFUSED, HEADS))     # F-path heads


def _paths(mb):
    """per-mb head->path map"""
    p = {}
    for h in range(HEADS):
        if h in DMA_H:
            p[h] = "D"
        elif h in FUSED_H:
            p[h] = "F"
            if _FRAC and h == HEADS - _NFUSED and mb % _FRAC == 0:
                p[h] = "A"
        else:
            p[h] = "A"
    return p

_CACHE = {}


def _register_gat_prelu():
    """Register a fused prelu(in0 + in1 + s0) custom DVE op (idempotent)."""
    import concourse.dve_ops as dops
    from concourse.dve_spec import Spec, Src0, Src1, C0, C2, maxx, lower
    from concourse.dve_spec import _has_src1
    from concourse.dve_uop import DveOpSpec

    name = "GAT_PRELU_ANT"
    if name in dops._SUB_OPCODE_FOR_NAME:
        for op in dops.OPS:
            if op.name == name:
                return op
        raise RuntimeError("GAT_PRELU_ANT row taken but op missing")

    def _ref(in0, in1, s0, s1, imm2):
        y = in0.astype(np.float32) + in1 + s0
        return np.maximum(y, y * imm2)

    y = Src0 + Src1 + C0
    spec = Spec(body=maxx(y, y * C2), reference=_ref)
    row = dops._CUSTOM_DVE_ROW_BASE + len(dops.OPS)
    shas = {}
    for ver in ("v3", "v4"):
        try:
            tmp = DveOpSpec(name=name, opcode=row, uops=lower(spec, ver=ver),
                            rd1_en=_has_src1(spec))
            shas[ver] = tmp.sha(ver)
        except Exception:
            pass
    op = dops.DveOp(name, spec, subdim=False, uops_sha=shas)
    dops.OPS.append(op)
    dops._SUB_OPCODE_FOR_NAME[name] = row
    dops.CUSTOM_DVE_SPECS[name] = spec
    return op


def _build():
    import concourse.bass as bass
    import concourse.mybir as mybir
    import concourse.tile as tile
    from concourse import bacc

    gat_prelu = _register_gat_prelu()

    f32 = mybir.dt.float32
    bf16 = mybir.dt.bfloat16
    Alu = mybir.AluOpType
    Act = mybir.ActivationFunctionType

    nc = bacc.Bacc("TRN2", target_bir_lowering=False, debug=False,
                   num_devices=NCORES)

    # host-prearranged [128, MB, x] layouts for straight contiguous DMA
    whb_d = nc.dram_tensor("whb", [128, MB, HO], bf16, kind="ExternalInput")
    bigat_d = nc.dram_tensor("bigat", [128, MB, NS], bf16,
                             kind="ExternalInput")
    srow = nc.dram_tensor("srow", [HEADS, NS], bf16, kind="ExternalInput")
    tpack = nc.dram_tensor("tpack", [128, MB, HEADS], f32,
                           kind="ExternalInput")
    outT = nc.dram_tensor("out", [HO, NS], f32, kind="ExternalOutput")

    nD = len(DMA_H)

    with tile.TileContext(nc) as tc:
        import contextlib
        with contextlib.ExitStack() as ctx:
            P1 = ctx.enter_context(tc.tile_pool(name="persist", bufs=1))
            xp = ctx.enter_context(tc.tile_pool(name="xp", bufs=3))
            pp = ctx.enter_context(tc.tile_pool(name="pp", bufs=3))
            iop = ctx.enter_context(tc.tile_pool(name="iop", bufs=2))
            hpp = ctx.enter_context(
                tc.tile_pool(name="hpp", bufs=1, space="PSUM"))

            alph = P1.tile([128, 1], f32)
            nc.vector.memset(alph, ALPHA)

            # ---- upfront loads ----
            sbc = P1.tile([128, HEADS, NS], bf16)   # s_h[n] bcast over parts
            srow_ap = srow.ap()
            sb_b = bass.AP(tensor=srow_ap.tensor, offset=srow_ap.offset,
                           ap=[[0, 128]] + srow_ap.ap)
            nc.sync.dma_start(out=sbc, in_=sb_b)
            tsb = P1.tile([128, MB, HEADS], f32)    # t_h[m] per partition
            nc.sync.dma_start(out=tsb, in_=tpack.ap())

            whb = P1.tile([128, MB, HO], bf16)
            bigat = P1.tile([128, MB, NS], bf16)
            CH = 4
            for q in range(MB // CH):
                cs = slice(CH * q, CH * (q + 1))
                nc.sync.dma_start(out=bigat[:, cs, :],
                                  in_=bigat_d.ap()[:, cs, :])
                nc.sync.dma_start(out=whb[:, cs, :],
                                  in_=whb_d.ap()[:, cs, :])

            # ---- PSUM accumulators: h'^T[(h,o), n], 2 heads per bank ----
            hp0 = hpp.tile([128, NS], f32, tag="hp0")
            hp1 = hpp.tile([128, NS], f32, tag="hp1")
            hp2 = hpp.tile([128, NS], f32, tag="hp2")
            hp3 = hpp.tile([128, NS], f32, tag="hp3")
            hps = [hp0, hp1, hp2, hp3]

            srow_b = srow.ap()[0:nD, :]
            for mb in range(MB):
                paths = _paths(mb)
                a_heads = [h for h in range(HEADS) if paths[h] == "A"]
                f_heads = [h for h in range(HEADS) if paths[h] == "F"]
                sl = bigat[:, mb, :]

                def bcast(k):
                    return bass.AP(tensor=sl.tensor, offset=sl.offset,
                                   ap=[sl.ap[0], [0, k], sl.ap[-1]])

                # D-path: DMA writes s rows (128-way bcast), then software-DGE
                # DMA accumulates this mb's bigat block on top (page-bcast)
                if nD:
                    xcd = xp.tile([128, nD, NS], bf16, tag="xcd")
                    nc.gpsimd.dma_start(
                        out=xcd,
                        in_=bass.AP(tensor=srow_b.tensor, offset=srow_b.offset,
                                    ap=[[0, 128]] + srow_b.ap))
                    bsl = bigat_d.ap()[:, mb, :]
                    nc.gpsimd.dma_start(
                        out=xcd,
                        in_=bass.AP(tensor=bsl.tensor, offset=bsl.offset,
                                    ap=[bsl.ap[0], [0, nD], bsl.ap[-1]]),
                        accum_op=Alu.add)
                # A-path s-adds, one batched TT
                nA = len(a_heads)
                if nA:
                    xcf = xp.tile([128, 4, NS], bf16, tag="xc")
                    xc = xcf[:, 0:nA, :]
                    sb_sl = sbc[:, a_heads[0], :]
                    nc.vector.tensor_tensor(
                        out=xc, in0=bcast(nA),
                        in1=bass.AP(tensor=sb_sl.tensor, offset=sb_sl.offset,
                                    ap=[sb_sl.ap[0], [NS, nA], sb_sl.ap[-1]]),
                        op=Alu.add)
                pc = pp.tile([128, HEADS, NS], bf16, tag="pc")
                # F-path: fused prelu(bigat + s + t) on DVE
                for hh in f_heads:
                    nc.vector._custom_dve(
                        gat_prelu, out=pc[:, hh, :], in0=sl,
                        in1=sbc[:, hh, :], s0=tsb[:, mb, hh:hh + 1],
                        imm2=ALPHA)
                # ACT: prelu with t bias for D- and A-path heads
                for i, hh in enumerate(DMA_H):
                    nc.scalar.activation(pc[:, hh, :], xcd[:, i, :],
                                         Act.Prelu,
                                         bias=tsb[:, mb, hh:hh + 1],
                                         scale=1.0, alpha=alph[:, 0:1])
                for i, hh in enumerate(a_heads):
                    nc.scalar.activation(pc[:, hh, :], xc[:, i, :],
                                         Act.Prelu,
                                         bias=tsb[:, mb, hh:hh + 1],
                                         scale=1.0, alpha=alph[:, 0:1])
                # PE: attention matmuls, fused heads first
                for hh in f_heads + DMA_H + a_heads:
                    po = 64 * (hh % 2)
                    nc.tensor.matmul(
                        hps[hh // 2][po:po + 64, :],
                        whb[:, mb, OUT_F * hh:OUT_F * (hh + 1)],
                        pc[:, hh, :],
                        start=(mb == 0), stop=(mb == MB - 1),
                        skip_group_check=True)

            # ---- output: elu, store transposed (host untransposes) ----
            for q in range(4):
                rpos = iop.tile([128, NS], f32, tag="rpos")
                nc.scalar.activation(rpos, hps[q], Act.Relu)
                rneg = iop.tile([128, NS], f32, tag="rneg")
                nc.scalar.activation(rneg, hps[q], Act.Relu, scale=-1.0)
                ex = iop.tile([128, NS], f32, tag="ex")
                nc.scalar.activation(ex, rneg, Act.Exp, scale=-1.0)
                oo = iop.tile([128, NS], f32, tag="oo")
                nc.vector.scalar_tensor_tensor(
                    out=oo, in0=rpos, scalar=-1.0, in1=ex,
                    op0=Alu.add, op1=Alu.add)
                nc.sync.dma_start(out=outT.ap()[128 * q:128 * (q + 1), :],
                                  in_=oo)

    nc.compile()
    return nc


def _prep_inputs(h, adj, W, a):
    bf = ml_dtypes.bfloat16
    # Wh[h, n, o] then column-major (h,o) concat -> [n, 64h+o]
    Wh = np.matmul(h[None, :, :], W)                       # [H, N, O] f32
    whb_no = Wh.transpose(1, 0, 2).reshape(N, HO)          # [N, HO]
    whb = np.ascontiguousarray(
        whb_no.reshape(MB, 128, HO).transpose(1, 0, 2)).astype(bf)
    a1 = a[:, :OUT_F, 0]                                   # [H, O] (s side)
    a2 = a[:, OUT_F:, 0]                                   # [H, O] (t side)
    s_full = np.matmul(Wh, a1[:, :, None])[:, :, 0]        # [H, N]
    t_full = np.matmul(Wh, a2[:, :, None])[:, :, 0]        # [H, N]
    tpack = np.ascontiguousarray(
        t_full.T.reshape(MB, 128, HEADS).transpose(1, 0, 2)).astype(
            np.float32)                                    # [128, MB, H]
    bigaT = ((adj.T.astype(np.float32) - 1.0) * BIG).astype(bf)  # [m, n]

    in_maps = []
    for c in range(NCORES):
        rows = slice(c * NS, (c + 1) * NS)
        bslice = np.ascontiguousarray(
            bigaT[:, rows].reshape(MB, 128, NS).transpose(1, 0, 2))
        in_maps.append({
            "whb": whb,
            "bigat": bslice,
            "srow": np.ascontiguousarray(s_full[:, rows]).astype(bf),
            "tpack": tpack,
        })
    return in_maps


def _get_nc():
    if "nc" not in _CACHE:
        _CACHE["nc"] = _build()
    return _CACHE["nc"]


def kernel(h, adj, W, a, _trace=False, _trace_kwargs=None):
    from concourse.bass_utils import run_bass_kernel_spmd

    h = np.asarray(h, dtype=np.float32)
    adj = np.asarray(adj, dtype=np.int32)
    W = np.asarray(W, dtype=np.float32)
    a = np.asarray(a, dtype=np.float32)

    nc = _get_nc()
    in_maps = _prep_inputs(h, adj, W, a)
    res = run_bass_kernel_spmd(nc, in_maps, core_ids=list(range(NCORES)),
                               trace=_trace, **(_trace_kwargs or {}))
    out = np.empty((N, HO), dtype=np.float32)
    for c in range(NCORES):
        out[c * NS:(c + 1) * NS, :] = res.results[c]["out"].T
    if _trace:
        _CACHE["last_results"] = res
    return out


# revision 10
# speedup vs baseline: 1.0941x; 1.0941x over previous
"""Multi-head graph attention layer (GAT, no softmax) on 8 Trainium2 NeuronCores.

Strategy: row-shard the N=4096 nodes across the 8 cores (512 rows each).
Host precomputes Wh = h@W (bf16, [m, (h,o)] layout), the attention
projections s_h[n] = Wh.a1, t_h[m] = Wh.a2, and the transposed additive
mask bigatT[m, n] = BIG*(adj[n, m]-1) (masked entries prelu to -0.2*BIG
~= -9.007e15, matching the reference's -9e15 to ~8e-4).

Device per core, per 128-row m-block:
    P^T[m, n] = prelu_0.2(bigatT[m, n] + s[n] + t[m])   per head
    h'^T[(h,o), n] += Wh[m, (h,o)]^T @ P^T[m, n]        (PSUM accumulate)
    out = elu(h'^T)

The N^2*H elementwise work is split across three engines:
  - heads ACT_H: DVE/Pool computes xc = bigatT + s (broadcast add), ACT
    applies Prelu with per-partition bias t.
  - heads FUSED_H: one custom fused DVE op computes the whole chain
    prelu(bigatT + s + t) in a single 1-elem/cycle instruction.
"""

import numpy as np
import ml_dtypes

N = 4096
IN_F = 512
OUT_F = 64
HEADS = 8
NCORES = 8
NS = N // NCORES          # 512 rows per core
MB = N // 128             # 32 m-blocks
HO = HEADS * OUT_F        # 512
BIG = float(np.float32(1.25 * 2.0**55))   # 0.2*BIG = 2^53 ~= 9.007e15
ALPHA = 0.2

# head-assignment knobs.  Head h on m-block mb takes one of three paths:
#   "F"  — fused custom DVE op does bigat+s+t+prelu in one instruction
#   "A"  — DVE batched tensor_tensor s-add, then ACT prelu with bias=t
#   "D"  — DMA writes s-broadcast + accumulates bigat (software DGE),
#          then ACT prelu with bias=t (zero DVE cost)
import os
_NDMAS = int(os.environ.get("GAT_NDMAS", "2"))    # heads with s-add via DMA
_NFUSED = int(os.environ.get("GAT_NFUSED", "4"))  # heads on fused DVE op
_FRAC = int(os.environ.get("GAT_FRAC", "4"))      # last fused head does "A"
#                                                   every 1-in-_FRAC mb (0=off)
DMA_H = list(range(_NDMAS))                       # D-path heads
FUSED_H = list(range(HEADS - _NFUSED, HEADS))     # F-path heads


def _paths(mb):
    """per-mb head->path map"""
    p = {}
    for h in range(HEADS):
        if h in DMA_H:
            p[h] = "D"
        elif h in FUSED_H:
            p[h] = "F"
            if _FRAC and h == HEADS - _NFUSED and mb % _FRAC == 0:
                p[h] = "A"
        else:
            p[h] = "A"
    return p

_CACHE = {}


def _register_gat_prelu():
    """Register a fused prelu(in0 + in1 + s0) custom DVE op (idempotent)."""
    import concourse.dve_ops as dops
    from concourse.dve_spec import Spec, Src0, Src1, C0, C2, maxx, lower
    from concourse.dve_spec import _has_src1
    from concourse.dve_uop import DveOpSpec

    name = "GAT_PRELU_ANT"
    if name in dops._SUB_OPCODE_FOR_NAME:
        for op in dops.OPS:
            if op.name == name:
                return op
        raise RuntimeError("GAT_PRELU_ANT row taken but op missing")

    def _ref(in0, in1, s0, s1, imm2):
        y = in0.astype(np.float32) + in1 + s0
        return np.maximum(y, y * imm2)

    y = Src0 + Src1 + C0
    spec = Spec(body=maxx(y, y * C2), reference=_ref)
    row = dops._CUSTOM_DVE_ROW_BASE + len(dops.OPS)
    shas = {}
    for ver in ("v3", "v4"):
        try:
            tmp = DveOpSpec(name=name, opcode=row, uops=lower(spec, ver=ver),
                            rd1_en=_has_src1(spec))
            shas[ver] = tmp.sha(ver)
        except Exception:
            pass
    op = dops.DveOp(name, spec, subdim=False, uops_sha=shas)
    dops.OPS.append(op)
    dops._SUB_OPCODE_FOR_NAME[name] = row
    dops.CUSTOM_DVE_SPECS[name] = spec
    return op


def _build():
    import concourse.bass as bass
    import concourse.mybir as mybir
    import concourse.tile as tile
    from concourse import bacc

    gat_prelu = _register_gat_prelu()

    f32 = mybir.dt.float32
    bf16 = mybir.dt.bfloat16
    Alu = mybir.AluOpType
    Act = mybir.ActivationFunctionType

    nc = bacc.Bacc("TRN2", target_bir_lowering=False, debug=False,
                   num_devices=NCORES)

    # host-prearranged [128, MB, x] layouts for straight contiguous DMA
    whb_d = nc.dram_tensor("whb", [128, MB, HO], bf16, kind="ExternalInput")
    bigat_d = nc.dram_tensor("bigat", [128, MB, NS], bf16,
                             kind="ExternalInput")
    srow = nc.dram_tensor("srow", [HEADS, NS], bf16, kind="ExternalInput")
    tpack = nc.dram_tensor("tpack", [128, MB, HEADS], f32,
                           kind="ExternalInput")
    outT = nc.dram_tensor("out", [HO, NS], f32, kind="ExternalOutput")

    nD = len(DMA_H)

    with tile.TileContext(nc) as tc:
        import contextlib
        with contextlib.ExitStack() as ctx:
            P1 = ctx.enter_context(tc.tile_pool(name="persist", bufs=1))
            xp = ctx.enter_context(tc.tile_pool(name="xp", bufs=3))
            xdp = ctx.enter_context(tc.tile_pool(name="xdp", bufs=8))
            pp = ctx.enter_context(tc.tile_pool(name="pp", bufs=3))
            iop = ctx.enter_context(tc.tile_pool(name="iop", bufs=2))
            hpp = ctx.enter_context(
                tc.tile_pool(name="hpp", bufs=1, space="PSUM"))

            alph = P1.tile([128, 1], f32)
            nc.vector.memset(alph, ALPHA)

            # ---- upfront loads ----
            sbc = P1.tile([128, HEADS, NS], bf16)   # s_h[n] bcast over parts
            srow_ap = srow.ap()
            sb_b = bass.AP(tensor=srow_ap.tensor, offset=srow_ap.offset,
                           ap=[[0, 128]] + srow_ap.ap)
            nc.sync.dma_start(out=sbc, in_=sb_b)
            tsb = P1.tile([128, MB, HEADS], f32)    # t_h[m] per partition
            nc.sync.dma_start(out=tsb, in_=tpack.ap())

            whb = P1.tile([128, MB, HO], bf16)
            bigat = P1.tile([128, MB, NS], bf16)
            CH = 4
            for q in range(MB // CH):
                cs = slice(CH * q, CH * (q + 1))
                nc.sync.dma_start(out=bigat[:, cs, :],
                                  in_=bigat_d.ap()[:, cs, :])
                nc.sync.dma_start(out=whb[:, cs, :],
                                  in_=whb_d.ap()[:, cs, :])

            # ---- PSUM accumulators: h'^T[(h,o), n], 2 heads per bank ----
            hp0 = hpp.tile([128, NS], f32, tag="hp0")
            hp1 = hpp.tile([128, NS], f32, tag="hp1")
            hp2 = hpp.tile([128, NS], f32, tag="hp2")
            hp3 = hpp.tile([128, NS], f32, tag="hp3")
            hps = [hp0, hp1, hp2, hp3]

            srow_b = srow.ap()[0:nD, :]
            PRE = 6  # D-path DMA pairs issued this many mbs ahead
            xcds = {}

            def emit_dpath(m):
                # DMA writes s rows (128-way bcast), then software-DGE DMA
                # accumulates that mb's bigat block on top (page-bcast)
                xcd = xdp.tile([128, nD, NS], bf16, tag="xcd")
                nc.gpsimd.dma_start(
                    out=xcd,
                    in_=bass.AP(tensor=srow_b.tensor, offset=srow_b.offset,
                                ap=[[0, 128]] + srow_b.ap))
                bsl = bigat_d.ap()[:, m, :]
                nc.gpsimd.dma_start(
                    out=xcd,
                    in_=bass.AP(tensor=bsl.tensor, offset=bsl.offset,
                                ap=[bsl.ap[0], [0, nD], bsl.ap[-1]]),
                    accum_op=Alu.add)
                xcds[m] = xcd

            for mb in range(MB):
                paths = _paths(mb)
                a_heads = [h for h in range(HEADS) if paths[h] == "A"]
                f_heads = [h for h in range(HEADS) if paths[h] == "F"]
                sl = bigat[:, mb, :]

                def bcast(k):
                    return bass.AP(tensor=sl.tensor, offset=sl.offset,
                                   ap=[sl.ap[0], [0, k], sl.ap[-1]])

                if nD:
                    if mb == 0:
                        for m in range(min(PRE, MB)):
                            emit_dpath(m)
                    if mb + PRE < MB:
                        emit_dpath(mb + PRE)
                    xcd = xcds.pop(mb)
                # A-path s-adds, one batched TT
                nA = len(a_heads)
                if nA:
                    xcf = xp.tile([128, 4, NS], bf16, tag="xc")
                    xc = xcf[:, 0:nA, :]
                    sb_sl = sbc[:, a_heads[0], :]
                    nc.vector.tensor_tensor(
                        out=xc, in0=bcast(nA),
                        in1=bass.AP(tensor=sb_sl.tensor, offset=sb_sl.offset,
                                    ap=[sb_sl.ap[0], [NS, nA], sb_sl.ap[-1]]),
                        op=Alu.add)
                pc = pp.tile([128, HEADS, NS], bf16, tag="pc")
                # F-path: fused prelu(bigat + s + t) on DVE
                for hh in f_heads:
                    nc.vector._custom_dve(
                        gat_prelu, out=pc[:, hh, :], in0=sl,
                        in1=sbc[:, hh, :], s0=tsb[:, mb, hh:hh + 1],
                        imm2=ALPHA)
                # ACT: prelu with t bias for D- and A-path heads
                for i, hh in enumerate(DMA_H):
                    nc.scalar.activation(pc[:, hh, :], xcd[:, i, :],
                                         Act.Prelu,
                                         bias=tsb[:, mb, hh:hh + 1],
                                         scale=1.0, alpha=alph[:, 0:1])
                for i, hh in enumerate(a_heads):
                    nc.scalar.activation(pc[:, hh, :], xc[:, i, :],
                                         Act.Prelu,
                                         bias=tsb[:, mb, hh:hh + 1],
                                         scale=1.0, alpha=alph[:, 0:1])
                # PE: attention matmuls, fused heads first
                for hh in f_heads + DMA_H + a_heads:
                    po = 64 * (hh % 2)
                    nc.tensor.matmul(
                        hps[hh // 2][po:po + 64, :],
                        whb[:, mb, OUT_F * hh:OUT_F * (hh + 1)],
                        pc[:, hh, :],
                        start=(mb == 0), stop=(mb == MB - 1),
                        skip_group_check=True)

            # ---- output: elu, store transposed (host untransposes) ----
            for q in range(4):
                rpos = iop.tile([128, NS], f32, tag="rpos")
                nc.scalar.activation(rpos, hps[q], Act.Relu)
                rneg = iop.tile([128, NS], f32, tag="rneg")
                nc.scalar.activation(rneg, hps[q], Act.Relu, scale=-1.0)
                ex = iop.tile([128, NS], f32, tag="ex")
                nc.scalar.activation(ex, rneg, Act.Exp, scale=-1.0)
                oo = iop.tile([128, NS], f32, tag="oo")
                nc.vector.scalar_tensor_tensor(
                    out=oo, in0=rpos, scalar=-1.0, in1=ex,
                    op0=Alu.add, op1=Alu.add)
                nc.sync.dma_start(out=outT.ap()[128 * q:128 * (q + 1), :],
                                  in_=oo)

    nc.compile()
    return nc


def _prep_inputs(h, adj, W, a):
    bf = ml_dtypes.bfloat16
    # Wh[h, n, o] then column-major (h,o) concat -> [n, 64h+o]
    Wh = np.matmul(h[None, :, :], W)                       # [H, N, O] f32
    whb_no = Wh.transpose(1, 0, 2).reshape(N, HO)          # [N, HO]
    whb = np.ascontiguousarray(
        whb_no.reshape(MB, 128, HO).transpose(1, 0, 2)).astype(bf)
    a1 = a[:, :OUT_F, 0]                                   # [H, O] (s side)
    a2 = a[:, OUT_F:, 0]                                   # [H, O] (t side)
    s_full = np.matmul(Wh, a1[:, :, None])[:, :, 0]        # [H, N]
    t_full = np.matmul(Wh, a2[:, :, None])[:, :, 0]        # [H, N]
    tpack = np.ascontiguousarray(
        t_full.T.reshape(MB, 128, HEADS).transpose(1, 0, 2)).astype(
            np.float32)                                    # [128, MB, H]
    bigaT = ((adj.T.astype(np.float32) - 1.0) * BIG).astype(bf)  # [m, n]

    in_maps = []
    for c in range(NCORES):
        rows = slice(c * NS, (c + 1) * NS)
        bslice = np.ascontiguousarray(
            bigaT[:, rows].reshape(MB, 128, NS).transpose(1, 0, 2))
        in_maps.append({
            "whb": whb,
            "bigat": bslice,
            "srow": np.ascontiguousarray(s_full[:, rows]).astype(bf),
            "tpack": tpack,
        })
    return in_maps


def _get_nc():
    if "nc" not in _CACHE:
        _CACHE["nc"] = _build()
    return _CACHE["nc"]


def kernel(h, adj, W, a, _trace=False, _trace_kwargs=None):
    from concourse.bass_utils import run_bass_kernel_spmd

    h = np.asarray(h, dtype=np.float32)
    adj = np.asarray(adj, dtype=np.int32)
    W = np.asarray(W, dtype=np.float32)
    a = np.asarray(a, dtype=np.float32)

    nc = _get_nc()
    in_maps = _prep_inputs(h, adj, W, a)
    res = run_bass_kernel_spmd(nc, in_maps, core_ids=list(range(NCORES)),
                               trace=_trace, **(_trace_kwargs or {}))
    out = np.empty((N, HO), dtype=np.float32)
    for c in range(NCORES):
        out[c * NS:(c + 1) * NS, :] = res.results[c]["out"].T
    if _trace:
        _CACHE["last_results"] = res
    return out


# revision 18
# speedup vs baseline: 1.4481x; 1.3236x over previous
"""Multi-head graph attention layer (GAT, no softmax) on 8 Trainium2 NeuronCores.

Strategy: row-shard the N=4096 nodes across the 8 cores (512 rows each).
Host precomputes Wh = h@W (bf16, [m, (h,o)] layout), the attention
projections s_h[n] = Wh.a1, t_h[m] = Wh.a2, and the transposed additive
mask bigatT[m, n] = BIG*(adj[n, m]-1) (masked entries prelu to -0.2*BIG
~= -9.007e15, matching the reference's -9e15 to ~8e-4).

Device per core, per 128-row m-block:
    P^T[m, n] = prelu_0.2(bigatT[m, n] + s[n] + t[m])   per head
    h'^T[(h,o), n] += Wh[m, (h,o)]^T @ P^T[m, n]        (PSUM accumulate)
    out = elu(h'^T)

The N^2*H elementwise work is split across three engines:
  - heads ACT_H: DVE/Pool computes xc = bigatT + s (broadcast add), ACT
    applies Prelu with per-partition bias t.
  - heads FUSED_H: one custom fused DVE op computes the whole chain
    prelu(bigatT + s + t) in a single 1-elem/cycle instruction.
"""

import numpy as np
import ml_dtypes

N = 4096
IN_F = 512
OUT_F = 64
HEADS = 8
NCORES = 8
NS = N // NCORES          # 512 rows per core
MB = N // 128             # 32 m-blocks
HO = HEADS * OUT_F        # 512
BIG = float(np.float32(1.25 * 2.0**55))   # 0.2*BIG = 2^53 ~= 9.007e15
ALPHA = 0.2

# head-assignment knobs.  Head h on m-block mb takes one of three paths:
#   "F"  — fused custom DVE op does bigat+s+t+prelu in one instruction
#   "A"  — DVE batched tensor_tensor s-add, then ACT prelu with bias=t
#   "D"  — DMA writes s-broadcast + accumulates bigat (software DGE),
#          then ACT prelu with bias=t (zero DVE cost)
import os
_NDMAS = int(os.environ.get("GAT_NDMAS", "0"))    # heads with s-add via DMA
_NFUSED = int(os.environ.get("GAT_NFUSED", "4"))  # heads on fused DVE op
_FRAC = int(os.environ.get("GAT_FRAC", "0"))      # last fused head does "A"
#                                                   every 1-in-_FRAC mb (0=off)
DMA_H = list(range(_NDMAS))                       # D-path heads
FUSED_H = list(range(HEADS - _NFUSED, HEADS))     # F-path heads


def _paths(mb):
    """per-mb head->path map"""
    p = {}
    for h in range(HEADS):
        if h in DMA_H:
            p[h] = "D"
        elif h in FUSED_H:
            p[h] = "F"
            if _FRAC and h == HEADS - _NFUSED and mb % _FRAC == 0:
                p[h] = "A"
        else:
            p[h] = "A"
    return p

_CACHE = {}


def _register_op(name, spec, subdim):
    """Register a custom DVE op (idempotent), computing its uops sha."""
    import concourse.dve_ops as dops
    from concourse.dve_spec import lower, _has_src1
    from concourse.dve_uop import DveOpSpec

    if name in dops._SUB_OPCODE_FOR_NAME:
        for op in dops.OPS:
            if op.name == name:
                return op
        raise RuntimeError(f"{name} row taken but op missing")
    row = dops._CUSTOM_DVE_ROW_BASE + len(dops.OPS)
    shas = {}
    for ver in ("v3", "v4"):
        try:
            tmp = DveOpSpec(name=name, opcode=row, uops=lower(spec, ver=ver),
                            rd1_en=_has_src1(spec))
            shas[ver] = tmp.sha(ver)
        except Exception:
            pass
    op = dops.DveOp(name, spec, subdim=subdim, uops_sha=shas)
    dops.OPS.append(op)
    dops._SUB_OPCODE_FOR_NAME[name] = row
    dops.CUSTOM_DVE_SPECS[name] = spec
    return op


def _register_gat_prelu():
    """Fused prelu(in0 + in1 + s0) with slope imm2, one head-block."""
    from concourse.dve_spec import Spec, Src0, Src1, C0, C2, maxx

    def _ref(in0, in1, s0, s1, imm2):
        y = in0.astype(np.float32) + in1 + s0
        return np.maximum(y, y * imm2)

    y = Src0 + Src1 + C0
    return _register_op("GAT_PRELU_ANT", Spec(body=maxx(y, y * C2), reference=_ref),
                        subdim=False)


def _register_gat_prelu2():
    """Paired fused prelu over [P, 2, N] pages: page s gets bias s0 + s*s1
    (s0 = t of head A, s1 = t_B - t_A), slope imm2.  in1 is the concatenated
    s-rows of both heads as a flat [P, 2N] stream."""
    from concourse.dve_spec import Spec, Src0, Src1, C0, C1, C2, PageIdx, maxx

    def _ref(in0, in1, s0, s1, imm2):
        P = in0.shape[0]
        x0 = in0.astype(np.float32).reshape(P, 2, -1)
        x1 = in1.astype(np.float32).reshape(P, 2, -1)
        s0 = np.asarray(s0, np.float32).reshape(P, 1, 1)
        s1 = np.asarray(s1, np.float32).reshape(P, 1, 1)
        t = s0 + np.arange(2, dtype=np.float32)[None, :, None] * s1
        y = x0 + x1 + t
        return np.maximum(y, y * imm2).reshape(in0.shape)

    y = Src0 + Src1 + PageIdx(C0, C1)
    return _register_op("GAT_PRELU2_ANT",
                        Spec(body=maxx(y, y * C2), reference=_ref),
                        subdim=True)


def _build():
    import concourse.bass as bass
    import concourse.mybir as mybir
    import concourse.tile as tile
    from concourse import bacc

    gat_prelu = _register_gat_prelu()
    gat_prelu2 = _register_gat_prelu2()

    f32 = mybir.dt.float32
    bf16 = mybir.dt.bfloat16
    Alu = mybir.AluOpType
    Act = mybir.ActivationFunctionType

    nc = bacc.Bacc("TRN2", target_bir_lowering=False, debug=False,
                   num_devices=NCORES)

    # host-prearranged [128, MB, x] layouts for straight contiguous DMA
    whb_d = nc.dram_tensor("whb", [128, MB, HO], bf16, kind="ExternalInput")
    bigat_d = nc.dram_tensor("bigat", [128, MB, NS], bf16,
                             kind="ExternalInput")
    srow = nc.dram_tensor("srow", [HEADS, NS], bf16, kind="ExternalInput")
    tpack = nc.dram_tensor("tpack", [128, MB, HEADS], f32,
                           kind="ExternalInput")
    tdelta = nc.dram_tensor("tdelta", [128, MB, HEADS // 2], f32,
                            kind="ExternalInput")
    outT = nc.dram_tensor("out", [HO, NS], f32, kind="ExternalOutput")

    nD = len(DMA_H)

    with tile.TileContext(nc) as tc:
        import contextlib
        with contextlib.ExitStack() as ctx:
            P1 = ctx.enter_context(tc.tile_pool(name="persist", bufs=1))
            xp = ctx.enter_context(tc.tile_pool(name="xp", bufs=3))
            xdp = ctx.enter_context(tc.tile_pool(name="xdp", bufs=8))
            pp = ctx.enter_context(tc.tile_pool(name="pp", bufs=3))
            iop = ctx.enter_context(tc.tile_pool(name="iop", bufs=2))
            hpp = ctx.enter_context(
                tc.tile_pool(name="hpp", bufs=1, space="PSUM"))

            alph = P1.tile([128, 1], f32)
            nc.vector.memset(alph, ALPHA)

            # ---- upfront loads ----
            sbc = P1.tile([128, HEADS, NS], bf16)   # s_h[n] bcast over parts
            srow_ap = srow.ap()
            sb_b = bass.AP(tensor=srow_ap.tensor, offset=srow_ap.offset,
                           ap=[[0, 128]] + srow_ap.ap)
            nc.sync.dma_start(out=sbc, in_=sb_b)
            tsb = P1.tile([128, MB, HEADS], f32)    # t_h[m] per partition
            nc.sync.dma_start(out=tsb, in_=tpack.ap())
            tdl = P1.tile([128, MB, HEADS // 2], f32)  # t_{h+1}-t_h per pair
            nc.sync.dma_start(out=tdl, in_=tdelta.ap())

            whb = P1.tile([128, MB, HO], bf16)
            bigat = P1.tile([128, MB, NS], bf16)
            CH = 4
            for q in range(MB // CH):
                cs = slice(CH * q, CH * (q + 1))
                nc.sync.dma_start(out=bigat[:, cs, :],
                                  in_=bigat_d.ap()[:, cs, :])
                nc.sync.dma_start(out=whb[:, cs, :],
                                  in_=whb_d.ap()[:, cs, :])

            # ---- PSUM accumulators: h'^T[(h,o), n], 2 heads per bank ----
            hp0 = hpp.tile([128, NS], f32, tag="hp0")
            hp1 = hpp.tile([128, NS], f32, tag="hp1")
            hp2 = hpp.tile([128, NS], f32, tag="hp2")
            hp3 = hpp.tile([128, NS], f32, tag="hp3")
            hps = [hp0, hp1, hp2, hp3]

            srow_b = srow.ap()[0:nD, :]
            PRE = 6  # D-path DMA pairs issued this many mbs ahead
            xcds = {}

            def emit_dpath(m):
                # DMA writes s rows (128-way bcast), then software-DGE DMA
                # accumulates that mb's bigat block on top (page-bcast)
                xcd = xdp.tile([128, nD, NS], bf16, tag="xcd")
                nc.gpsimd.dma_start(
                    out=xcd,
                    in_=bass.AP(tensor=srow_b.tensor, offset=srow_b.offset,
                                ap=[[0, 128]] + srow_b.ap))
                bsl = bigat_d.ap()[:, m, :]
                nc.gpsimd.dma_start(
                    out=xcd,
                    in_=bass.AP(tensor=bsl.tensor, offset=bsl.offset,
                                ap=[bsl.ap[0], [0, nD], bsl.ap[-1]]),
                    accum_op=Alu.add)
                xcds[m] = xcd

            for mb in range(MB):
                paths = _paths(mb)
                a_heads = [h for h in range(HEADS) if paths[h] == "A"]
                f_heads = [h for h in range(HEADS) if paths[h] == "F"]
                sl = bigat[:, mb, :]

                def bcast(k):
                    return bass.AP(tensor=sl.tensor, offset=sl.offset,
                                   ap=[sl.ap[0], [0, k], sl.ap[-1]])

                if nD:
                    if mb == 0:
                        for m in range(min(PRE, MB)):
                            emit_dpath(m)
                    if mb + PRE < MB:
                        emit_dpath(mb + PRE)
                    xcd = xcds.pop(mb)
                # A-path s-adds, one batched TT
                nA = len(a_heads)
                if nA:
                    xcf = xp.tile([128, 4, NS], bf16, tag="xc")
                    xc = xcf[:, 0:nA, :]
                    sb_sl = sbc[:, a_heads[0], :]
                    nc.vector.tensor_tensor(
                        out=xc, in0=bcast(nA),
                        in1=bass.AP(tensor=sb_sl.tensor, offset=sb_sl.offset,
                                    ap=[sb_sl.ap[0], [NS, nA], sb_sl.ap[-1]]),
                        op=Alu.add)
                pc = pp.tile([128, HEADS, NS], bf16, tag="pc")
                # F-path: fused prelu(bigat + s + t) on DVE.  Adjacent head
                # pairs go through the paged op (one instr per pair, t
                # page-interpolated); stragglers use the single-head op.
                fs = sorted(f_heads)
                i = 0
                while i < len(fs):
                    hh = fs[i]
                    if hh % 2 == 0 and i + 1 < len(fs) and fs[i + 1] == hh + 1:
                        sb_sl = sbc[:, hh, :]
                        nc.vector._custom_dve(
                            gat_prelu2, out=pc[:, hh:hh + 2, :],
                            in0=bcast(2),
                            in1=bass.AP(tensor=sb_sl.tensor,
                                        offset=sb_sl.offset,
                                        ap=[sb_sl.ap[0], [1, 2 * NS]]),
                            s0=tsb[:, mb, hh:hh + 1],
                            s1=tdl[:, mb, hh // 2:hh // 2 + 1],
                            imm2=ALPHA)
                        i += 2
                    else:
                        nc.vector._custom_dve(
                            gat_prelu, out=pc[:, hh, :], in0=sl,
                            in1=sbc[:, hh, :], s0=tsb[:, mb, hh:hh + 1],
                            imm2=ALPHA)
                        i += 1
                # ACT: prelu with t bias for D- and A-path heads
                for i, hh in enumerate(DMA_H):
                    nc.scalar.activation(pc[:, hh, :], xcd[:, i, :],
                                         Act.Prelu,
                                         bias=tsb[:, mb, hh:hh + 1],
                                         scale=1.0, alpha=alph[:, 0:1])
                for i, hh in enumerate(a_heads):
                    nc.scalar.activation(pc[:, hh, :], xc[:, i, :],
                                         Act.Prelu,
                                         bias=tsb[:, mb, hh:hh + 1],
                                         scale=1.0, alpha=alph[:, 0:1])
                # PE: attention matmuls, fused heads first
                for hh in f_heads + DMA_H + a_heads:
                    po = 64 * (hh % 2)
                    nc.tensor.matmul(
                        hps[hh // 2][po:po + 64, :],
                        whb[:, mb, OUT_F * hh:OUT_F * (hh + 1)],
                        pc[:, hh, :],
                        start=(mb == 0), stop=(mb == MB - 1),
                        skip_group_check=True)

            # ---- output: elu, store transposed (host untransposes) ----
            for q in range(4):
                rpos = iop.tile([128, NS], f32, tag="rpos")
                nc.scalar.activation(rpos, hps[q], Act.Relu)
                rneg = iop.tile([128, NS], f32, tag="rneg")
                nc.scalar.activation(rneg, hps[q], Act.Relu, scale=-1.0)
                ex = iop.tile([128, NS], f32, tag="ex")
                nc.scalar.activation(ex, rneg, Act.Exp, scale=-1.0)
                oo = iop.tile([128, NS], f32, tag="oo")
                nc.vector.scalar_tensor_tensor(
                    out=oo, in0=rpos, scalar=-1.0, in1=ex,
                    op0=Alu.add, op1=Alu.add)
                nc.sync.dma_start(out=outT.ap()[128 * q:128 * (q + 1), :],
                                  in_=oo)

    nc.compile()
    return nc


def _prep_inputs(h, adj, W, a):
    bf = ml_dtypes.bfloat16
    # Wh[h, n, o] then column-major (h,o) concat -> [n, 64h+o]
    Wh = np.matmul(h[None, :, :], W)                       # [H, N, O] f32
    whb_no = Wh.transpose(1, 0, 2).reshape(N, HO)          # [N, HO]
    whb = np.ascontiguousarray(
        whb_no.reshape(MB, 128, HO).transpose(1, 0, 2)).astype(bf)
    a1 = a[:, :OUT_F, 0]                                   # [H, O] (s side)
    a2 = a[:, OUT_F:, 0]                                   # [H, O] (t side)
    s_full = np.matmul(Wh, a1[:, :, None])[:, :, 0]        # [H, N]
    t_full = np.matmul(Wh, a2[:, :, None])[:, :, 0]        # [H, N]
    tpack = np.ascontiguousarray(
        t_full.T.reshape(MB, 128, HEADS).transpose(1, 0, 2)).astype(
            np.float32)                                    # [128, MB, H]
    tdelta = np.ascontiguousarray(
        tpack[:, :, 1::2] - tpack[:, :, 0::2])             # [128, MB, H/2]
    bigaT = ((adj.T.astype(np.float32) - 1.0) * BIG).astype(bf)  # [m, n]

    in_maps = []
    for c in range(NCORES):
        rows = slice(c * NS, (c + 1) * NS)
        bslice = np.ascontiguousarray(
            bigaT[:, rows].reshape(MB, 128, NS).transpose(1, 0, 2))
        in_maps.append({
            "whb": whb,
            "bigat": bslice,
            "srow": np.ascontiguousarray(s_full[:, rows]).astype(bf),
            "tpack": tpack,
            "tdelta": tdelta,
        })
    return in_maps


def _get_nc():
    if "nc" not in _CACHE:
        _CACHE["nc"] = _build()
    return _CACHE["nc"]


def kernel(h, adj, W, a, _trace=False, _trace_kwargs=None):
    from concourse.bass_utils import run_bass_kernel_spmd

    h = np.asarray(h, dtype=np.float32)
    adj = np.asarray(adj, dtype=np.int32)
    W = np.asarray(W, dtype=np.float32)
    a = np.asarray(a, dtype=np.float32)

    nc = _get_nc()
    in_maps = _prep_inputs(h, adj, W, a)
    res = run_bass_kernel_spmd(nc, in_maps, core_ids=list(range(NCORES)),
                               trace=_trace, **(_trace_kwargs or {}))
    out = np.empty((N, HO), dtype=np.float32)
    for c in range(NCORES):
        out[c * NS:(c + 1) * NS, :] = res.results[c]["out"].T
    if _trace:
        _CACHE["last_results"] = res
    return out


# revision 22
# speedup vs baseline: 1.5894x; 1.0976x over previous
"""Multi-head graph attention layer (GAT, no softmax) on 8 Trainium2 NeuronCores.

Strategy: row-shard the N=4096 nodes across the 8 cores (512 rows each).
Host precomputes Wh = h@W (bf16, [m, (h,o)] layout), the attention
projections s_h[n] = Wh.a1, t_h[m] = Wh.a2, and the transposed additive
mask bigatT[m, n] = BIG*(adj[n, m]-1) (masked entries prelu to -0.2*BIG
~= -9.007e15, matching the reference's -9e15 to ~8e-4).

Device per core, per 128-row m-block:
    P^T[m, n] = prelu_0.2(bigatT[m, n] + s[n] + t[m])   per head
    h'^T[(h,o), n] += Wh[m, (h,o)]^T @ P^T[m, n]        (PSUM accumulate)
    out = elu(h'^T)

The N^2*H elementwise work is split across three engines:
  - heads ACT_H: DVE/Pool computes xc = bigatT + s (broadcast add), ACT
    applies Prelu with per-partition bias t.
  - heads FUSED_H: one custom fused DVE op computes the whole chain
    prelu(bigatT + s + t) in a single 1-elem/cycle instruction.
"""

import numpy as np
import ml_dtypes

N = 4096
IN_F = 512
OUT_F = 64
HEADS = 8
NCORES = 8
NS = N // NCORES          # 512 rows per core
MB = N // 128             # 32 m-blocks
HO = HEADS * OUT_F        # 512
BIG = float(np.float32(1.25 * 2.0**55))   # 0.2*BIG = 2^53 ~= 9.007e15
ALPHA = 0.2

# Heads 0.._NDVE-1 compute prelu(s+t) in one fused custom DVE instruction;
# the remaining heads use ACT Prelu with bias=t.  The additive-BIG mask is
# factored out entirely: P = bigat02 + prelu(s [+] t), and the bigat02 term
# goes through 4 shared full-width matmuls on the (underutilized) PE.
import os
_NDVE = int(os.environ.get("GAT_NDVE", "5"))

_CACHE = {}


def _register_op(name, spec, subdim):
    """Register a custom DVE op (idempotent), computing its uops sha."""
    import concourse.dve_ops as dops
    from concourse.dve_spec import lower, _has_src1
    from concourse.dve_uop import DveOpSpec

    if name in dops._SUB_OPCODE_FOR_NAME:
        for op in dops.OPS:
            if op.name == name:
                return op
        raise RuntimeError(f"{name} row taken but op missing")
    row = dops._CUSTOM_DVE_ROW_BASE + len(dops.OPS)
    shas = {}
    for ver in ("v3", "v4"):
        try:
            tmp = DveOpSpec(name=name, opcode=row, uops=lower(spec, ver=ver),
                            rd1_en=_has_src1(spec))
            shas[ver] = tmp.sha(ver)
        except Exception:
            pass
    op = dops.DveOp(name, spec, subdim=subdim, uops_sha=shas)
    dops.OPS.append(op)
    dops._SUB_OPCODE_FOR_NAME[name] = row
    dops.CUSTOM_DVE_SPECS[name] = spec
    return op


def _register_gat_prelu():
    """Fused prelu(in0 + in1 + s0) with slope imm2, one head-block."""
    from concourse.dve_spec import Spec, Src0, Src1, C0, C2, maxx

    def _ref(in0, in1, s0, s1, imm2):
        y = in0.astype(np.float32) + in1 + s0
        return np.maximum(y, y * imm2)

    y = Src0 + Src1 + C0
    return _register_op("GAT_PRELU_ANT", Spec(body=maxx(y, y * C2), reference=_ref),
                        subdim=False)


def _register_gat_prelu2():
    """Paired fused prelu over [P, 2, N] pages: page s gets bias s0 + s*s1
    (s0 = t of head A, s1 = t_B - t_A), slope imm2.  in1 is the concatenated
    s-rows of both heads as a flat [P, 2N] stream."""
    from concourse.dve_spec import Spec, Src0, Src1, C0, C1, C2, PageIdx, maxx

    def _ref(in0, in1, s0, s1, imm2):
        P = in0.shape[0]
        x0 = in0.astype(np.float32).reshape(P, 2, -1)
        x1 = in1.astype(np.float32).reshape(P, 2, -1)
        s0 = np.asarray(s0, np.float32).reshape(P, 1, 1)
        s1 = np.asarray(s1, np.float32).reshape(P, 1, 1)
        t = s0 + np.arange(2, dtype=np.float32)[None, :, None] * s1
        y = x0 + x1 + t
        return np.maximum(y, y * imm2).reshape(in0.shape)

    y = Src0 + Src1 + PageIdx(C0, C1)
    return _register_op("GAT_PRELU2_ANT",
                        Spec(body=maxx(y, y * C2), reference=_ref),
                        subdim=True)


def _register_gat_prelu_e():
    """prelu(in0 + in1) with slope s0: computes prelu_0.2(s[n] + t[m]) for a
    multi-head block, with in1 the per-head t column broadcast along the
    free dim.  Two tensor streams, one scalar slope."""
    from concourse.dve_spec import Spec, Src0, Src1, C0, maxx

    def _ref(in0, in1, s0, s1, imm2):
        y = in0.astype(np.float32) + in1
        return np.maximum(y, y * s0)

    y = Src0 + Src1
    return _register_op("GAT_PRELU_E_ANT",
                        Spec(body=maxx(y, y * C0), reference=_ref),
                        subdim=False)


def _build():
    import concourse.bass as bass
    import concourse.mybir as mybir
    import concourse.tile as tile
    from concourse import bacc

    gat_prelu_e = _register_gat_prelu_e()

    f32 = mybir.dt.float32
    bf16 = mybir.dt.bfloat16
    Alu = mybir.AluOpType
    Act = mybir.ActivationFunctionType

    nc = bacc.Bacc("TRN2", target_bir_lowering=False, debug=False,
                   num_devices=NCORES)

    # host-prearranged [128, MB, x] layouts for straight contiguous DMA.
    # bigat is pre-scaled by the prelu slope: {-0.2*BIG, 0}.
    whb_d = nc.dram_tensor("whb", [128, MB, HO], bf16, kind="ExternalInput")
    bigat_d = nc.dram_tensor("bigat", [128, MB, NS], bf16,
                             kind="ExternalInput")
    srow = nc.dram_tensor("srow", [HEADS, NS], bf16, kind="ExternalInput")
    tpack = nc.dram_tensor("tpack", [128, MB, HEADS], f32,
                           kind="ExternalInput")
    outT = nc.dram_tensor("out", [HO, NS], f32, kind="ExternalOutput")

    nDV = _NDVE                 # heads 0..nDV-1 on the fused DVE op
    act_heads = list(range(nDV, HEADS))

    with tile.TileContext(nc) as tc:
        import contextlib
        with contextlib.ExitStack() as ctx:
            P1 = ctx.enter_context(tc.tile_pool(name="persist", bufs=1))
            pp = ctx.enter_context(tc.tile_pool(name="pp", bufs=3))
            iop = ctx.enter_context(tc.tile_pool(name="iop", bufs=2))
            hpp = ctx.enter_context(
                tc.tile_pool(name="hpp", bufs=1, space="PSUM"))

            alph = P1.tile([128, 1], f32)
            nc.vector.memset(alph, ALPHA)

            # ---- upfront loads ----
            sbc = P1.tile([128, HEADS, NS], bf16)   # s_h[n] bcast over parts
            srow_ap = srow.ap()
            sb_b = bass.AP(tensor=srow_ap.tensor, offset=srow_ap.offset,
                           ap=[[0, 128]] + srow_ap.ap)
            nc.sync.dma_start(out=sbc, in_=sb_b)
            tsb = P1.tile([128, MB, HEADS], f32)    # t_h[m] per partition
            nc.sync.dma_start(out=tsb, in_=tpack.ap())

            whb = P1.tile([128, MB, HO], bf16)
            bigat = P1.tile([128, MB, NS], bf16)
            # small chunks first so mb0 deps land fast
            CHUNKS = [1, 1, 1, 1, 4, 4, 4, 4, 4, 4, 4]
            pos = 0
            for ch in CHUNKS:
                cs = slice(pos, pos + ch)
                nc.sync.dma_start(out=bigat[:, cs, :],
                                  in_=bigat_d.ap()[:, cs, :])
                nc.sync.dma_start(out=whb[:, cs, :],
                                  in_=whb_d.ap()[:, cs, :])
                pos += ch

            # ---- PSUM accumulators: h'^T[(h,o), n], 2 heads per bank ----
            hp0 = hpp.tile([128, NS], f32, tag="hp0")
            hp1 = hpp.tile([128, NS], f32, tag="hp1")
            hp2 = hpp.tile([128, NS], f32, tag="hp2")
            hp3 = hpp.tile([128, NS], f32, tag="hp3")
            hps = [hp0, hp1, hp2, hp3]

            for mb in range(MB):
                # shared mask matmuls: hps[q] (+)= whb_cols_q^T @ bigat02
                for q in range(4):
                    nc.tensor.matmul(
                        hps[q], whb[:, mb, 128 * q:128 * (q + 1)],
                        bigat[:, mb, :],
                        start=(mb == 0), stop=False,
                        skip_group_check=True)
                pc = pp.tile([128, HEADS, NS], bf16, tag="pc")
                # DVE: fused prelu(s + t) for heads 0..nDV-1, one instruction
                # (in1 = t columns broadcast along the free dim)
                tsl = tsb[:, mb, 0:nDV]
                nc.vector._custom_dve(
                    gat_prelu_e, out=pc[:, 0:nDV, :],
                    in0=sbc[:, 0:nDV, :],
                    in1=bass.AP(tensor=tsl.tensor, offset=tsl.offset,
                                ap=[tsl.ap[0], [tsl.ap[-1][0], nDV],
                                    [0, NS]]),
                    s0=ALPHA)
                # ACT: prelu(s + t) via bias for the rest
                for hh in act_heads:
                    nc.scalar.activation(pc[:, hh, :], sbc[:, hh, :],
                                         Act.Prelu,
                                         bias=tsb[:, mb, hh:hh + 1],
                                         scale=1.0, alpha=alph[:, 0:1])
                # PE: per-head attention matmuls (accumulate onto mask term)
                for hh in list(range(nDV)) + act_heads:
                    po = 64 * (hh % 2)
                    nc.tensor.matmul(
                        hps[hh // 2][po:po + 64, :],
                        whb[:, mb, OUT_F * hh:OUT_F * (hh + 1)],
                        pc[:, hh, :],
                        start=False, stop=(mb == MB - 1),
                        skip_group_check=True)

            # ---- output: elu, store transposed (host untransposes).
            # q0/q1 chains lean on ACT, q2/q3 on DVE, so the tails overlap.
            for q in range(4):
                if q < 2:
                    rpos = iop.tile([128, NS], f32, tag=f"rpos{q}")
                    nc.scalar.activation(rpos, hps[q], Act.Relu)
                    rneg = iop.tile([128, NS], f32, tag=f"rneg{q}")
                    nc.scalar.activation(rneg, hps[q], Act.Relu, scale=-1.0)
                    ex = iop.tile([128, NS], f32, tag=f"ex{q}")
                    nc.scalar.activation(ex, rneg, Act.Exp, scale=-1.0)
                else:
                    rpos = iop.tile([128, NS], f32, tag=f"rpos{q}")
                    nc.vector.tensor_scalar(rpos, hps[q], 0.0, None, Alu.max)
                    rneg = iop.tile([128, NS], f32, tag=f"rneg{q}")
                    nc.vector.tensor_scalar(rneg, hps[q], 0.0, None, Alu.min)
                    ex = iop.tile([128, NS], f32, tag=f"ex{q}")
                    nc.scalar.activation(ex, rneg, Act.Exp)
                oo = iop.tile([128, NS], f32, tag=f"oo{q}")
                nc.vector.scalar_tensor_tensor(
                    out=oo, in0=rpos, scalar=-1.0, in1=ex,
                    op0=Alu.add, op1=Alu.add)
                nc.sync.dma_start(out=outT.ap()[128 * q:128 * (q + 1), :],
                                  in_=oo)

    nc.compile()
    return nc


def _prep_inputs(h, adj, W, a):
    bf = ml_dtypes.bfloat16
    # Wh[h, n, o] then column-major (h,o) concat -> [n, 64h+o]
    Wh = np.matmul(h[None, :, :], W)                       # [H, N, O] f32
    whb_no = Wh.transpose(1, 0, 2).reshape(N, HO)          # [N, HO]
    whb = np.ascontiguousarray(
        whb_no.reshape(MB, 128, HO).transpose(1, 0, 2)).astype(bf)
    a1 = a[:, :OUT_F, 0]                                   # [H, O] (s side)
    a2 = a[:, OUT_F:, 0]                                   # [H, O] (t side)
    s_full = np.matmul(Wh, a1[:, :, None])[:, :, 0]        # [H, N]
    t_full = np.matmul(Wh, a2[:, :, None])[:, :, 0]        # [H, N]
    tpack = np.ascontiguousarray(
        t_full.T.reshape(MB, 128, HEADS).transpose(1, 0, 2)).astype(
            np.float32)                                    # [128, MB, H]
    # mask pre-scaled by the prelu slope: {-0.2*BIG, 0}
    bigaT = ((adj.T.astype(np.float32) - 1.0) * (ALPHA * BIG)).astype(bf)

    in_maps = []
    for c in range(NCORES):
        rows = slice(c * NS, (c + 1) * NS)
        bslice = np.ascontiguousarray(
            bigaT[:, rows].reshape(MB, 128, NS).transpose(1, 0, 2))
        in_maps.append({
            "whb": whb,
            "bigat": bslice,
            "srow": np.ascontiguousarray(s_full[:, rows]).astype(bf),
            "tpack": tpack,
        })
    return in_maps


def _get_nc():
    if "nc" not in _CACHE:
        _CACHE["nc"] = _build()
    return _CACHE["nc"]


def kernel(h, adj, W, a, _trace=False, _trace_kwargs=None):
    from concourse.bass_utils import run_bass_kernel_spmd

    h = np.asarray(h, dtype=np.float32)
    adj = np.asarray(adj, dtype=np.int32)
    W = np.asarray(W, dtype=np.float32)
    a = np.asarray(a, dtype=np.float32)

    nc = _get_nc()
    in_maps = _prep_inputs(h, adj, W, a)
    res = run_bass_kernel_spmd(nc, in_maps, core_ids=list(range(NCORES)),
                               trace=_trace, **(_trace_kwargs or {}))
    out = np.empty((N, HO), dtype=np.float32)
    for c in range(NCORES):
        out[c * NS:(c + 1) * NS, :] = res.results[c]["out"].T
    if _trace:
        _CACHE["last_results"] = res
    return out


# revision 27
# speedup vs baseline: 1.7925x; 1.1278x over previous
"""Multi-head graph attention layer (GAT, no softmax) on 8 Trainium2 NeuronCores.

Strategy: row-shard the N=4096 nodes across the 8 cores (512 rows each).
Host precomputes Wh = h@W (bf16, [m, (h,o)] layout), the attention
projections s_h[n] = Wh.a1, t_h[m] = Wh.a2, and the transposed additive
mask bigatT[m, n] = BIG*(adj[n, m]-1) (masked entries prelu to -0.2*BIG
~= -9.007e15, matching the reference's -9e15 to ~8e-4).

Device per core, per 128-row m-block:
    P^T[m, n] = prelu_0.2(bigatT[m, n] + s[n] + t[m])   per head
    h'^T[(h,o), n] += Wh[m, (h,o)]^T @ P^T[m, n]        (PSUM accumulate)
    out = elu(h'^T)

The N^2*H elementwise work is split across three engines:
  - heads ACT_H: DVE/Pool computes xc = bigatT + s (broadcast add), ACT
    applies Prelu with per-partition bias t.
  - heads FUSED_H: one custom fused DVE op computes the whole chain
    prelu(bigatT + s + t) in a single 1-elem/cycle instruction.
"""

import numpy as np
import ml_dtypes

N = 4096
IN_F = 512
OUT_F = 64
HEADS = 8
NCORES = 8
NS = N // NCORES          # 512 rows per core
MB = N // 128             # 32 m-blocks
HO = HEADS * OUT_F        # 512
BIG = float(np.float32(1.25 * 2.0**55))   # 0.2*BIG = 2^53 ~= 9.007e15
ALPHA = 0.2

# Heads 0.._NDVE-1 compute prelu(s+t) in one fused custom DVE instruction;
# the remaining heads use ACT Prelu with bias=t.  The additive-BIG mask is
# factored out entirely: P = bigat02 + prelu(s [+] t), and the bigat02 term
# goes through 4 shared full-width matmuls on the (underutilized) PE.
import os
_NDVE = int(os.environ.get("GAT_NDVE", "5"))

_CACHE = {}


def _register_op(name, spec, subdim):
    """Register a custom DVE op (idempotent), computing its uops sha."""
    import concourse.dve_ops as dops
    from concourse.dve_spec import lower, _has_src1
    from concourse.dve_uop import DveOpSpec

    if name in dops._SUB_OPCODE_FOR_NAME:
        for op in dops.OPS:
            if op.name == name:
                return op
        raise RuntimeError(f"{name} row taken but op missing")
    row = dops._CUSTOM_DVE_ROW_BASE + len(dops.OPS)
    shas = {}
    for ver in ("v3", "v4"):
        try:
            tmp = DveOpSpec(name=name, opcode=row, uops=lower(spec, ver=ver),
                            rd1_en=_has_src1(spec))
            shas[ver] = tmp.sha(ver)
        except Exception:
            pass
    op = dops.DveOp(name, spec, subdim=subdim, uops_sha=shas)
    dops.OPS.append(op)
    dops._SUB_OPCODE_FOR_NAME[name] = row
    dops.CUSTOM_DVE_SPECS[name] = spec
    return op


def _register_gat_prelu():
    """Fused prelu(in0 + in1 + s0) with slope imm2, one head-block."""
    from concourse.dve_spec import Spec, Src0, Src1, C0, C2, maxx

    def _ref(in0, in1, s0, s1, imm2):
        y = in0.astype(np.float32) + in1 + s0
        return np.maximum(y, y * imm2)

    y = Src0 + Src1 + C0
    return _register_op("GAT_PRELU_ANT", Spec(body=maxx(y, y * C2), reference=_ref),
                        subdim=False)


def _register_gat_prelu2():
    """Paired fused prelu over [P, 2, N] pages: page s gets bias s0 + s*s1
    (s0 = t of head A, s1 = t_B - t_A), slope imm2.  in1 is the concatenated
    s-rows of both heads as a flat [P, 2N] stream."""
    from concourse.dve_spec import Spec, Src0, Src1, C0, C1, C2, PageIdx, maxx

    def _ref(in0, in1, s0, s1, imm2):
        P = in0.shape[0]
        x0 = in0.astype(np.float32).reshape(P, 2, -1)
        x1 = in1.astype(np.float32).reshape(P, 2, -1)
        s0 = np.asarray(s0, np.float32).reshape(P, 1, 1)
        s1 = np.asarray(s1, np.float32).reshape(P, 1, 1)
        t = s0 + np.arange(2, dtype=np.float32)[None, :, None] * s1
        y = x0 + x1 + t
        return np.maximum(y, y * imm2).reshape(in0.shape)

    y = Src0 + Src1 + PageIdx(C0, C1)
    return _register_op("GAT_PRELU2_ANT",
                        Spec(body=maxx(y, y * C2), reference=_ref),
                        subdim=True)


def _register_gat_prelu_e():
    """prelu(in0 + in1) with slope s0: computes prelu_0.2(s[n] + t[m]) for a
    multi-head block, with in1 the per-head t column broadcast along the
    free dim.  Two tensor streams, one scalar slope."""
    from concourse.dve_spec import Spec, Src0, Src1, C0, maxx

    def _ref(in0, in1, s0, s1, imm2):
        y = in0.astype(np.float32) + in1
        return np.maximum(y, y * s0)

    y = Src0 + Src1
    return _register_op("GAT_PRELU_E_ANT",
                        Spec(body=maxx(y, y * C0), reference=_ref),
                        subdim=False)


def _build():
    import concourse.bass as bass
    import concourse.mybir as mybir
    import concourse.tile as tile
    from concourse import bacc

    gat_prelu_e = _register_gat_prelu_e()

    f32 = mybir.dt.float32
    bf16 = mybir.dt.bfloat16
    Alu = mybir.AluOpType
    Act = mybir.ActivationFunctionType

    nc = bacc.Bacc("TRN2", target_bir_lowering=False, debug=False,
                   num_devices=NCORES)

    # host-prearranged [128, MB, x] layouts for straight contiguous DMA.
    # bigat is pre-scaled by the prelu slope: {-0.2*BIG, 0}.
    whb_d = nc.dram_tensor("whb", [128, MB, HO], bf16, kind="ExternalInput")
    bigat_d = nc.dram_tensor("bigat", [128, MB, NS], bf16,
                             kind="ExternalInput")
    srow = nc.dram_tensor("srow", [HEADS, NS], bf16, kind="ExternalInput")
    tpack = nc.dram_tensor("tpack", [128, MB, HEADS], f32,
                           kind="ExternalInput")
    tpackh = nc.dram_tensor("tpackh", [128, MB, HEADS], bf16,
                            kind="ExternalInput")
    outT = nc.dram_tensor("out", [HO, NS], f32, kind="ExternalOutput")

    nDV = _NDVE                 # heads 0..nDV-1 on the fused DVE op
    act_heads = list(range(nDV, HEADS))

    with tile.TileContext(nc) as tc:
        import contextlib
        with contextlib.ExitStack() as ctx:
            P1 = ctx.enter_context(tc.tile_pool(name="persist", bufs=1))
            pp = ctx.enter_context(tc.tile_pool(name="pp", bufs=4))
            iop = ctx.enter_context(tc.tile_pool(name="iop", bufs=2))
            hpp = ctx.enter_context(
                tc.tile_pool(name="hpp", bufs=1, space="PSUM"))

            alph = P1.tile([128, 1], f32)
            nc.vector.memset(alph, ALPHA)

            # ---- upfront loads ----
            tsb = P1.tile([128, MB, HEADS], f32)    # t_h[m] per partition
            nc.sync.dma_start(out=tsb, in_=tpack.ap())
            tsbh = P1.tile([128, MB, HEADS], bf16)  # same in bf16 (DVE in1)
            nc.sync.dma_start(out=tsbh, in_=tpackh.ap())
            # s rows: load once into one partition, broadcast to all 128
            # partitions via a rank-1 ones matmul (much faster than a
            # 128-way broadcast DMA)
            srow1 = P1.tile([1, HEADS * NS], bf16)
            sr_ap = srow.ap()
            nc.sync.dma_start(
                out=srow1,
                in_=bass.AP(tensor=sr_ap.tensor, offset=sr_ap.offset,
                            ap=[[HEADS * NS, 1], [1, HEADS * NS]]))
            ones1 = P1.tile([1, 128], bf16)
            nc.vector.memset(ones1, 1.0)
            sbc = P1.tile([128, HEADS, NS], bf16)   # s_h[n] bcast over parts
            with tc.tile_pool(name="bcp", bufs=2, space="PSUM") as bcp:
                for sg in range(HEADS):
                    sps = bcp.tile([128, NS], f32, tag="sps")
                    nc.tensor.matmul(sps, ones1,
                                     srow1[:, NS * sg:NS * (sg + 1)],
                                     start=True, stop=True)
                    nc.vector.tensor_copy(sbc[:, sg, :], sps)

            whb = P1.tile([128, MB, HO], bf16)
            bigat = P1.tile([128, MB, NS], bf16)
            # small chunks first so mb0 deps land fast
            CHUNKS = [1, 1, 1, 1, 4, 4, 4, 4, 4, 4, 4]
            pos = 0
            for ch in CHUNKS:
                cs = slice(pos, pos + ch)
                nc.sync.dma_start(out=bigat[:, cs, :],
                                  in_=bigat_d.ap()[:, cs, :])
                nc.sync.dma_start(out=whb[:, cs, :],
                                  in_=whb_d.ap()[:, cs, :])
                pos += ch

            # ---- PSUM accumulators: h'^T[(h,o), n], 2 heads per bank ----
            hp0 = hpp.tile([128, NS], f32, tag="hp0")
            hp1 = hpp.tile([128, NS], f32, tag="hp1")
            hp2 = hpp.tile([128, NS], f32, tag="hp2")
            hp3 = hpp.tile([128, NS], f32, tag="hp3")
            hps = [hp0, hp1, hp2, hp3]

            for mb in range(MB):
                # shared mask matmuls: hps[q] (+)= whb_cols_q^T @ bigat02
                for q in range(4):
                    nc.tensor.matmul(
                        hps[q], whb[:, mb, 128 * q:128 * (q + 1)],
                        bigat[:, mb, :],
                        start=(mb == 0), stop=False,
                        skip_group_check=True)
                pc = pp.tile([128, HEADS, NS], bf16, tag="pc")
                # DVE: fused prelu(s + t) for heads 0..nDV-1, one instruction
                # (in1 = t columns broadcast along the free dim)
                tsl = tsbh[:, mb, 0:nDV]
                nc.vector._custom_dve(
                    gat_prelu_e, out=pc[:, 0:nDV, :],
                    in0=sbc[:, 0:nDV, :],
                    in1=bass.AP(tensor=tsl.tensor, offset=tsl.offset,
                                ap=[tsl.ap[0], [tsl.ap[-1][0], nDV],
                                    [0, NS]]),
                    s0=ALPHA)
                # ACT: prelu(s + t) via bias for the rest
                for hh in act_heads:
                    nc.scalar.activation(pc[:, hh, :], sbc[:, hh, :],
                                         Act.Prelu,
                                         bias=tsb[:, mb, hh:hh + 1],
                                         scale=1.0, alpha=alph[:, 0:1])
                # PE: per-head attention matmuls (accumulate onto mask term)
                for hh in list(range(nDV)) + act_heads:
                    po = 64 * (hh % 2)
                    nc.tensor.matmul(
                        hps[hh // 2][po:po + 64, :],
                        whb[:, mb, OUT_F * hh:OUT_F * (hh + 1)],
                        pc[:, hh, :],
                        start=False, stop=(mb == MB - 1),
                        skip_group_check=True)

            # ---- output: elu, store transposed (host untransposes).
            # q0/q1 chains lean on ACT, q2/q3 on DVE, so the tails overlap.
            for q in range(4):
                rpos = iop.tile([128, NS], f32, tag=f"rpos{q}")
                rneg = iop.tile([128, NS], f32, tag=f"rneg{q}")
                ex = iop.tile([128, NS], f32, tag=f"ex{q}")
                if q < 2:
                    nc.scalar.activation(rpos, hps[q], Act.Relu)
                    nc.scalar.activation(rneg, hps[q], Act.Relu, scale=-1.0)
                    nc.scalar.activation(ex, rneg, Act.Exp, scale=-1.0)
                else:
                    nc.vector.tensor_scalar(rpos, hps[q], 0.0, None, Alu.max)
                    nc.vector.tensor_scalar(rneg, hps[q], 0.0, None, Alu.min)
                    nc.scalar.activation(ex, rneg, Act.Exp)
                oo = iop.tile([128, NS], f32, tag=f"oo{q}")
                nc.vector.scalar_tensor_tensor(
                    out=oo, in0=rpos, scalar=-1.0, in1=ex,
                    op0=Alu.add, op1=Alu.add)
                nc.sync.dma_start(out=outT.ap()[128 * q:128 * (q + 1), :],
                                  in_=oo)

    nc.compile()
    return nc


def _prep_inputs(h, adj, W, a):
    bf = ml_dtypes.bfloat16
    # Wh[h, n, o] then column-major (h,o) concat -> [n, 64h+o]
    Wh = np.matmul(h[None, :, :], W)                       # [H, N, O] f32
    whb_no = Wh.transpose(1, 0, 2).reshape(N, HO)          # [N, HO]
    whb = np.ascontiguousarray(
        whb_no.reshape(MB, 128, HO).transpose(1, 0, 2)).astype(bf)
    a1 = a[:, :OUT_F, 0]                                   # [H, O] (s side)
    a2 = a[:, OUT_F:, 0]                                   # [H, O] (t side)
    s_full = np.matmul(Wh, a1[:, :, None])[:, :, 0]        # [H, N]
    t_full = np.matmul(Wh, a2[:, :, None])[:, :, 0]        # [H, N]
    tpack = np.ascontiguousarray(
        t_full.T.reshape(MB, 128, HEADS).transpose(1, 0, 2)).astype(
            np.float32)                                    # [128, MB, H]
    # mask pre-scaled by the prelu slope: {-0.2*BIG, 0}
    bigaT = ((adj.T.astype(np.float32) - 1.0) * (ALPHA * BIG)).astype(bf)

    in_maps = []
    for c in range(NCORES):
        rows = slice(c * NS, (c + 1) * NS)
        bslice = np.ascontiguousarray(
            bigaT[:, rows].reshape(MB, 128, NS).transpose(1, 0, 2))
        in_maps.append({
            "whb": whb,
            "bigat": bslice,
            "srow": np.ascontiguousarray(s_full[:, rows]).astype(bf),
            "tpack": tpack,
            "tpackh": tpack.astype(bf),
        })
    return in_maps


def _get_nc():
    if "nc" not in _CACHE:
        _CACHE["nc"] = _build()
    return _CACHE["nc"]


def kernel(h, adj, W, a, _trace=False, _trace_kwargs=None):
    from concourse.bass_utils import run_bass_kernel_spmd

    h = np.asarray(h, dtype=np.float32)
    adj = np.asarray(adj, dtype=np.int32)
    W = np.asarray(W, dtype=np.float32)
    a = np.asarray(a, dtype=np.float32)

    nc = _get_nc()
    in_maps = _prep_inputs(h, adj, W, a)
    res = run_bass_kernel_spmd(nc, in_maps, core_ids=list(range(NCORES)),
                               trace=_trace, **(_trace_kwargs or {}))
    out = np.empty((N, HO), dtype=np.float32)
    for c in range(NCORES):
        out[c * NS:(c + 1) * NS, :] = res.results[c]["out"].T
    if _trace:
        _CACHE["last_results"] = res
    return out
